# revision 24
# baseline (speedup 1.0000x reference)
"""DeepTermRankingListNet Trainium2 kernel.

Full-input contract: kernel(**inputs) takes the unsharded numpy inputs and
returns the full [1, 256] output. Internally shards candidates C=256 across
8 NeuronCores (32 each), replicates the embedding table + small params,
runs one SPMD Bass/Tile kernel via run_bass_kernel_spmd, and concatenates
the per-core [32] outputs.

v5: replace the 17 serialized indirect_dma_start gathers (994ns+ fixed
SWDGE ucode overhead EACH -> ~24us) with the batched InstDMAGatherAnt
ucode path:
  stage 1: 4 dma_gather window calls (int16 idx limit -> 4-row blocks,
    idx = row//4 < 32768 per 131072-row window) pull 4-row blocks into a
    staging tile S[:, win][128, 4, NCAP] (transposed: partition=d).
    Data-dependent window counts ride in via reg_load'd num_idxs_reg.
  stage 2: 2 SBUF-source dma_gather calls re-gather single columns of S
    (elem = one 256B column = one table row, transposed) -> BT[128, 2048]
    in candidate-pair checkerboard order. idx = col index of S, <16384.
  A's 64 rows keep one indirect_dma_start (128 rows, 1 instruction).
Pool desc-gen total ~7us instead of ~24us.

Compute restructured in the transposed (position-major) domain:
  TT[pos,k] = tanh(B @ AM^T) via per-chunk matmul(lhsT=BT_chunk, rhs=AMT);
  rows-numerators R_T[k,c] via matmul(lhsT=TT_chunk, rhs=mask2) (PE, no
  wide DVE reduce); cols-numerators EC via cheap 64-col DVE reduce;
  BG (for the LT-weighted newB matmul) via PE transposes of BT chunks.
Final combine identical to v4 (bf16 pipeline, fp32 string branch with
DVE Newton rsqrt, exp/tanh on one ACT table set).
"""

import numpy as np

V, D, K, C, DS = 500000, 128, 64, 256, 200
NCORES = 8
CC = C // NCORES   # 32 candidates per core
NP = CC // 2       # 16 candidate-pair chunks
NPOS = CC * K      # 2048 gathered B rows per core
GAMMA = 0.5

RB = 4                  # rows per gathered block (idx = row//RB fits int16)
NBLK = V // RB          # 125000 blocks
WSZ = 32768             # blocks per window (int16 ucode index limit)
NWIN = (NBLK + WSZ - 1) // WSZ  # 4
NCAP = 640              # per-window position capacity (mean ~537, +5 sigma);
                        # windows are dup-padded to exactly NCAP valid idxs
SWCOLS = RB * NCAP      # S cols per window (j-major [RB, NCAP])
SCOLS = NWIN * SWCOLS   # 16384

# hx int16 layout (per 128 partitions; idx data replicated per 16-row group)
HX_S1 = 0         # 4 windows x (NCAP/16 = 40) cols
HX_S2 = 160       # 2 halves x 64 cols
HX_AIDX = 288     # 1 int32 col (int16 cols 288..290): A row ids, 128 parts
HX_W = 290

_BUILT = None


def _build_nc():
    import concourse.bacc as bacc
    import concourse.mybir as mybir
    from concourse import bass
    from concourse.tile import TileContext

    f32 = mybir.dt.float32
    bf16 = mybir.dt.bfloat16
    i16 = mybir.dt.int16
    i32 = mybir.dt.int32
    AF = mybir.ActivationFunctionType
    ALU = mybir.AluOpType
    AX = mybir.AxisListType

    nc = bacc.Bacc("TRN2", debug=False)

    table_d = nc.dram_tensor("table", (V, D), bf16, kind="ExternalInput")
    hx_d = nc.dram_tensor("hx", (128, HX_W), i16, kind="ExternalInput")
    # packed bf16 params: att | ident | w  -> [128, 384]
    pk_d = nc.dram_tensor("pk", (128, 3 * 128), bf16, kind="ExternalInput")
    # packed fp32 smalls: str1 | str2 | b -> [CC, 2*DS+1]
    sm_d = nc.dram_tensor("sm", (CC, 2 * DS + 1), f32, kind="ExternalInput")
    y_d = nc.dram_tensor("y", (CC, 1), f32, kind="ExternalOutput")

    with TileContext(nc) as tc:
        with (
            tc.tile_pool(name="pers", bufs=1) as pp,
            tc.tile_pool(name="ps_tt", bufs=2, space="PSUM") as ps_tt,
            tc.tile_pool(name="ps_bg", bufs=2, space="PSUM") as ps_bg,
            tc.tile_pool(name="ps_sm", bufs=2, space="PSUM") as ps_sm,
            tc.tile_pool(name="ps_acc", bufs=1, space="PSUM") as ps_acc,
        ):
            # ---- persistent SBUF tiles ----
            hx_sb = pp.tile([128, HX_W], i16, tag="hx")
            pk_sb = pp.tile([128, 3 * 128], bf16, tag="pk")
            att_sb = pk_sb[:, 0:128]
            ident = pk_sb[:, 128:256]
            w_sb = pk_sb[:, 256:384]
            sm_sb = pp.tile([CC, 2 * DS + 1], f32, tag="sm")
            str1_sb = sm_sb[:, 0:DS]
            str2_sb = sm_sb[:, DS : 2 * DS]
            b_sb = sm_sb[:, 2 * DS : 2 * DS + 1]

            S = pp.tile([128, SCOLS], bf16, tag="s")      # staged 4-row blocks
            BT = pp.tile([128, NPOS], bf16, tag="bt")     # B^T checkerboard
            AG = pp.tile([128, 128], bf16, tag="ag")      # t1 rows (dup 2x)
            BGs = pp.tile([128, NPOS], bf16, tag="bgs")   # per-chunk B rows
            TT_sb = pp.tile([128, NP * K], bf16, tag="tt")
            ECall = pp.tile([128, NP], f32, tag="ec")
            LT = pp.tile([128, CC], bf16, tag="lt")
            ET2 = pp.tile([K, CC], bf16, tag="et2")
            mask2 = pp.tile([128, 2], bf16, tag="mask2")
            VBT_sb = pp.tile([128, CC], f32, tag="vbt")
            PZ_sb = pp.tile([128, CC], bf16, tag="pz")

            A_T_sb = pp.tile([128, K], bf16, tag="at")
            AMT_sb = pp.tile([128, K], bf16, tag="amt")
            AW_sb = pp.tile([K, 128], bf16, tag="aw")

            ones128 = pp.tile([128, 1], f32, tag="ones")
            ones128b = pp.tile([128, 1], bf16, tag="onesb128")
            ones64b = pp.tile([K, 1], bf16, tag="onesb")
            scr200 = pp.tile([CC, DS], f32, tag="scr200")
            s12_sb = pp.tile([CC, 1], f32, tag="s12")
            s2_sb2 = pp.tile([CC, 1], f32, tag="s2c")
            r12_sb = pp.tile([CC, 1], f32, tag="r12")
            dot_sb = pp.tile([CC, 1], f32, tag="dot")
            ssq2_sb = pp.tile([CC, 1], f32, tag="ssq2")
            ssq1_sb = pp.tile([CC, 1], f32, tag="ssq1")
            den2_sb = pp.tile([CC, 1], f32, tag="den2")
            den_sb = pp.tile([CC, 1], f32, tag="den")
            rden_sb = pp.tile([CC, 1], f32, tag="rden")
            strs_sb = pp.tile([CC, 1], f32, tag="strs")
            sbh_sb = pp.tile([CC, 1], f32, tag="sbh")
            nwt = pp.tile([CC, 1], f32, tag="nwt")
            y_sb = pp.tile([CC, 1], f32, tag="y")

            # ---- input DMAs (hx first: the gather stream waits on it) ----
            nc.sync.dma_start(out=hx_sb[:, :], in_=hx_d[:, :])
            nc.scalar.dma_start(out=pk_sb[:, :], in_=pk_d[:, :])
            nc.sync.dma_start(out=sm_sb[:, :], in_=sm_d[:, :])

            aidx_ap = hx_sb[:, HX_AIDX : HX_AIDX + 2].bitcast(i32)

            # ---- gather stream on Pool: A first (gates A-prep), then the
            # 4 window block-gathers, then the 2 stage-2 column re-gathers ----
            nc.gpsimd.indirect_dma_start(
                out=AG[:, :],
                out_offset=None,
                in_=table_d[:, :],
                in_offset=bass.IndirectOffsetOnAxis(ap=aidx_ap, axis=0),
            )

            for w in range(NWIN):
                nblkw = WSZ if w < NWIN - 1 else NBLK - (NWIN - 1) * WSZ
                # transpose=False: element i (a 4-row 1KB block) lands whole
                # in partition i%128, col-block i//128 of this window region
                nc.gpsimd.dma_gather(
                    S[:, w * SWCOLS : (w + 1) * SWCOLS].rearrange(
                        "p (b e) -> p b e", e=RB * D
                    ),
                    table_d[w * WSZ * RB : (w * WSZ + nblkw) * RB, :].rearrange(
                        "(b r) d -> b (r d)", r=RB
                    ),
                    hx_sb[:, 40 * w : 40 * (w + 1)],
                    NCAP,
                    NCAP,
                    RB * D,
                    elem_step=RB * D,
                    transpose=False,
                )

            # stage 2: re-gather single rows (256B from one partition of S,
            # rank-stripe addressed) transposed into BT columns.
            # num_idxs=1024 faults the SBUF-source ucode (probed: <=896 ok),
            # so split 2048 as 768+768+512.
            for off, n in ((0, 768), (768, 768), (1536, 512)):
                nc.gpsimd.dma_gather(
                    BT[:, off : off + n].rearrange("p (o i) -> p o i", i=n),
                    S[:, :],
                    hx_sb[:, HX_S2 + off // 16 : HX_S2 + (off + n) // 16],
                    n,
                    n,
                    D,
                    transpose=True,
                    sbuf_tokens_per_rank=128,
                    sbuf_free_dim_per_rank=2 * D,
                )

            # ---- constants ----
            nc.vector.memset(ones128[:, :], 1.0)
            nc.vector.memset(ones128b[:, :], 1.0)
            nc.vector.memset(ones64b[:, :], 1.0)
            nc.vector.memset(LT[:, :], 0.0)
            nc.vector.memset(mask2[:, :], 0.0)
            nc.vector.memset(mask2[0:64, 0:1], 1.0)
            nc.vector.memset(mask2[64:128, 1:2], 1.0)

            # ---- string branch: DVE only (exp/tanh stay the sole ACT set).
            # rden = rsqrt(|s1|^2|s2|^2) via prescale + Newton (v4). ----
            nc.vector.tensor_tensor(out=scr200[:, :], in0=str2_sb[:, :],
                                    in1=str1_sb[:, :], op=ALU.mult)
            nc.vector.reduce_sum(dot_sb[:, :], scr200[:, :], axis=AX.X)
            nc.vector.tensor_tensor(out=scr200[:, :], in0=str2_sb[:, :],
                                    in1=str2_sb[:, :], op=ALU.mult)
            nc.vector.reduce_sum(ssq2_sb[:, :], scr200[:, :], axis=AX.X)
            nc.vector.tensor_tensor(out=scr200[:, :], in0=str1_sb[:, :],
                                    in1=str1_sb[:, :], op=ALU.mult)
            nc.vector.reduce_sum(ssq1_sb[:, :], scr200[:, :], axis=AX.X)
            nc.vector.tensor_tensor(out=den2_sb[:, :], in0=ssq1_sb[:, :],
                                    in1=ssq2_sb[:, :], op=ALU.mult)
            SCL = 1.0 / 40000.0
            nc.vector.tensor_scalar(out=den_sb[:, :], in0=den2_sb[:, :],
                                    scalar1=SCL, scalar2=None, op0=ALU.mult)
            nc.vector.memset(rden_sb[:, :], 1.0)
            for _ in range(5):
                nc.vector.tensor_tensor(out=nwt[:, :], in0=rden_sb[:, :],
                                        in1=rden_sb[:, :], op=ALU.mult)
                nc.vector.tensor_tensor(out=nwt[:, :], in0=nwt[:, :],
                                        in1=den_sb[:, :], op=ALU.mult)
                nc.vector.tensor_scalar(out=nwt[:, :], in0=nwt[:, :],
                                        scalar1=-0.5, scalar2=1.5,
                                        op0=ALU.mult, op1=ALU.add)
                nc.vector.tensor_tensor(out=rden_sb[:, :], in0=rden_sb[:, :],
                                        in1=nwt[:, :], op=ALU.mult)
            nc.vector.tensor_scalar(out=rden_sb[:, :], in0=rden_sb[:, :],
                                    scalar1=1.0 / 200.0, scalar2=None,
                                    op0=ALU.mult)
            nc.vector.tensor_tensor(out=strs_sb[:, :], in0=dot_sb[:, :],
                                    in1=rden_sb[:, :], op=ALU.mult)
            nc.vector.tensor_scalar(out=sbh_sb[:, :], in0=strs_sb[:, :],
                                    scalar1=b_sb[:, 0:1], scalar2=GAMMA,
                                    op0=ALU.add, op1=ALU.mult)

            # ---- A prep: A_T = A^T; AMT = (A@att)^T; AW = A@W ----
            A_sb = AG[0:64, :]
            A_T_p = ps_sm.tile([128, K], bf16, tag="sm", bufs=2)
            nc.tensor.transpose(A_T_p[:, :], A_sb, ident[0:64, 0:64])
            nc.scalar.copy(A_T_sb[:, :], A_T_p[:, :])
            AMT_p = ps_sm.tile([128, K], f32, tag="sm", bufs=2)
            nc.tensor.matmul(AMT_p[:, :], lhsT=att_sb, rhs=A_T_sb[:, :],
                             start=True, stop=True)
            nc.scalar.copy(AMT_sb[:, :], AMT_p[:, :])
            AW_p = ps_sm.tile([K, 128], f32, tag="sm", bufs=2)
            nc.tensor.matmul(AW_p[:, :], lhsT=A_T_sb[:, :], rhs=w_sb,
                             start=True, stop=True)
            nc.scalar.copy(AW_sb[:, :], AW_p[:, :])

            # ---- persistent PSUM accumulators (one shared bank) ----
            # col layout: RT [0:32) (rows 0:64), VBT [32:64), T1u [64:96),
            # s1/s2/z cols 96/97/98 (rows 0:32)
            acc = ps_acc.tile([128, 128], f32, tag="acc", bufs=1)

            # ---- per-chunk pipeline over 16 candidate-pair chunks ----
            for t in range(NP):
                btc = BT[:, 128 * t : 128 * (t + 1)]
                ttc = TT_sb[:, K * t : K * (t + 1)]
                TT_p = ps_tt.tile([128, K], f32, tag="ttp", name="tt_p")
                nc.tensor.matmul(TT_p[:, :], lhsT=btc, rhs=AMT_sb[:, :],
                                 start=True, stop=True)
                nc.scalar.activation(ttc, TT_p[:, :], AF.Tanh)
                nc.vector.reduce_sum(ECall[:, t : t + 1], ttc, axis=AX.X)
                nc.tensor.matmul(acc[0:K, 2 * t : 2 * t + 2], lhsT=ttc,
                                 rhs=mask2[:, :], start=True, stop=True)
                BG_p = ps_bg.tile([128, 128], bf16, tag="bgp", name="bg_p")
                nc.tensor.transpose(BG_p[:, :], btc, ident)
                if t % 2 == 0:
                    nc.vector.tensor_copy(BGs[:, 128 * t : 128 * (t + 1)],
                                          BG_p[:, :])
                else:
                    nc.scalar.copy(BGs[:, 128 * t : 128 * (t + 1)], BG_p[:, :])

                if t in (5, 11, 15):
                    # cols-softmax weights for this tranche into the LT
                    # checkerboard, then its newB matmuls
                    b0 = t - 5 if t != 15 else 12
                    nb = t + 1 - b0
                    c0 = 2 * b0
                    nc.scalar.activation(LT[0:64, c0 : c0 + 2 * nb - 1 : 2],
                                         ECall[0:64, b0 : b0 + nb],
                                         AF.Exp, scale=1.0 / K)
                    nc.scalar.activation(LT[64:128, c0 + 1 : c0 + 2 * nb : 2],
                                         ECall[64:128, b0 : b0 + nb],
                                         AF.Exp, scale=1.0 / K)
                    for u in range(b0, t + 1):
                        nc.tensor.matmul(
                            acc[:, 32 + 2 * u : 32 + 2 * u + 2],
                            lhsT=BGs[:, 128 * u : 128 * (u + 1)],
                            rhs=LT[:, 2 * u : 2 * u + 2],
                            start=True, stop=True,
                        )

            # ---- rows weights + bilinear combine ----
            nc.scalar.activation(ET2[:, :], acc[0:K, 0:CC], AF.Exp,
                                 scale=1.0 / K)
            nc.tensor.matmul(acc[:, 64:96], lhsT=AW_sb[:, :], rhs=ET2[:, :],
                             start=True, stop=True)
            nc.tensor.matmul(acc[0:CC, 96:97], lhsT=ET2[:, :],
                             rhs=ones64b[:, :], start=True, stop=True)
            nc.tensor.matmul(acc[0:CC, 97:98], lhsT=LT[:, :],
                             rhs=ones128b[:, :], start=True, stop=True)
            nc.vector.tensor_scalar(out=s2_sb2[:, :], in0=acc[0:CC, 97:98],
                                    scalar1=1.0 / GAMMA, scalar2=None,
                                    op0=ALU.mult)
            nc.vector.tensor_tensor(out=s12_sb[:, :], in0=acc[0:CC, 96:97],
                                    in1=s2_sb2[:, :], op=ALU.mult)
            nc.vector.reciprocal(r12_sb[:, :], s12_sb[:, :])
            nc.vector.tensor_copy(VBT_sb[:, :], acc[:, 32:64])
            nc.vector.tensor_tensor(out=PZ_sb[:, :], in0=acc[:, 64:96],
                                    in1=VBT_sb[:, :], op=ALU.mult)
            nc.tensor.matmul(acc[0:CC, 98:99], lhsT=PZ_sb[:, :],
                             rhs=ones128b[:, :], start=True, stop=True)
            nc.vector.tensor_scalar(out=y_sb[:, :], in0=acc[0:CC, 98:99],
                                    scalar1=r12_sb[:, 0:1],
                                    scalar2=sbh_sb[:, 0:1],
                                    op0=ALU.mult, op1=ALU.add)

            nc.sync.dma_start(out=y_d[:, :], in_=y_sb[:, :])

    nc.compile()
    return nc


def get_nc():
    global _BUILT
    if _BUILT is None:
        _BUILT = _build_nc()
    return _BUILT


def _wrap16(vals: np.ndarray, cols: int, fill: int) -> np.ndarray:
    """Pack vals[i] at [i%16, i//16] of an int16 [16, cols] block."""
    out = np.full((16, cols), fill, np.int16)
    i = np.arange(len(vals))
    out[i % 16, i // 16] = vals.astype(np.int16)
    return out


def make_in_maps(table, str_t1, str_t2s, att_mat, W_bi, b_bi, t1_ctx, t2_ctx):
    import ml_dtypes

    table = np.asarray(table, dtype=np.float32)
    str_t1 = np.asarray(str_t1, dtype=np.float32).reshape(DS)
    str_t2s = np.asarray(str_t2s, dtype=np.float32)
    att_mat = np.asarray(att_mat, dtype=np.float32)
    w2d = np.asarray(W_bi, dtype=np.float32).reshape(D, D)
    bval = float(np.asarray(b_bi).reshape(-1)[0])
    t1 = np.asarray(t1_ctx).astype(np.int64)
    t2 = np.asarray(t2_ctx).astype(np.int64)

    table_bf = table.astype(ml_dtypes.bfloat16)
    pk = np.concatenate(
        [att_mat, np.eye(D, dtype=np.float32), w2d], axis=1
    ).astype(ml_dtypes.bfloat16)  # [128, 384]

    sm = np.empty((CC, 2 * DS + 1), np.float32)
    sm[:, 0:DS] = str_t1[None, :]
    sm[:, 2 * DS] = bval

    # checkerboard position order: pos p -> chunk t=p//128, q=p%128,
    # candidate 2t + (q>=64), ctx q%64
    pos = np.arange(NPOS)
    tchunk = pos // 128
    q = pos % 128
    cand = 2 * tchunk + (q >= 64)
    ctx = q % 64

    in_maps = []
    for ci in range(NCORES):
        t2c = t2[ci * CC : (ci + 1) * CC]          # [CC, K]
        rows = t2c[cand, ctx]                       # [NPOS] desired table rows
        blk = rows // RB
        sub = rows % RB
        win = (blk // WSZ).astype(np.int64)
        lblk = (blk - win * WSZ).astype(np.int64)

        hx = np.zeros((128, HX_W), np.int16)
        s2_idx = np.empty(NPOS, np.int64)
        for w in range(NWIN):
            sel = np.nonzero(win == w)[0]
            n_w = len(sel)
            assert 0 < n_w <= NCAP, f"window {w} overflow: {n_w} > {NCAP}"
            # dup-pad to exactly NCAP valid indices so the whole S region is
            # written (keeps every byte of the stage-2 source defined)
            lw = np.concatenate(
                [lblk[sel], np.full(NCAP - n_w, lblk[sel][-1], np.int64)]
            )
            blk16 = _wrap16(lw, NCAP // 16, -1)
            hx[:, HX_S1 + 40 * w : HX_S1 + 40 * (w + 1)] = np.tile(
                blk16, (8, 1)
            )
            # stage-2 source address in S: stage-1 put window rank r at
            # partition r%128, col-block r//128 (1KB per block, RB rows);
            # row j is the 256B stripe rank w*(NCAP//128)*RB + (r//128)*RB
            # + j at partition r%128 -> idx = rank_id*128 + (r%128)
            r = np.arange(n_w)
            rank_id = w * (NCAP // 128) * RB + (r // 128) * RB + sub[sel]
            s2_idx[sel] = rank_id * 128 + (r % 128)
        for off, n in ((0, 768), (768, 768), (1536, 512)):
            part16 = _wrap16(s2_idx[off : off + n], n // 16, -1)
            hx[:, HX_S2 + off // 16 : HX_S2 + (off + n) // 16] = np.tile(
                part16, (8, 1)
            )
        hx32 = hx.view(np.int32)
        aidx = np.concatenate([t1, t1]).astype(np.int32)  # [128]
        hx32[:, HX_AIDX // 2] = aidx

        smc = sm.copy()
        smc[:, DS : 2 * DS] = str_t2s[ci * CC : (ci + 1) * CC]
        in_maps.append({
            "table": table_bf,
            "hx": hx,
            "pk": pk,
            "sm": smc,
        })
    return in_maps


def run(inputs: dict, trace: bool = False):
    from concourse.bass_utils import run_bass_kernel_spmd

    nc = get_nc()
    in_maps = make_in_maps(**inputs)
    res = run_bass_kernel_spmd(
        nc, in_maps, core_ids=list(range(NCORES)), trace=trace
    )
    y = np.concatenate([r["y"].reshape(-1) for r in res.results])
    return y.reshape(1, C).astype(np.float32), res


def kernel(**inputs) -> np.ndarray:
    y, _ = run(inputs, trace=False)
    return y


# revision 25
# speedup vs baseline: 1.4321x; 1.4321x over previous
"""DeepTermRankingListNet Trainium2 kernel.

Full-input contract: kernel(**inputs) takes the unsharded numpy inputs and
returns the full [1, 256] output. Internally shards candidates C=256 across
8 NeuronCores (32 each), replicates the embedding table + small params,
runs one SPMD Bass/Tile kernel via run_bass_kernel_spmd, and concatenates
the per-core [32] outputs.

v6. The gather stream is v4's: 17 indirect_dma_start calls (A block first,
then 16 candidate-pair blocks, one row per partition), which probing showed
is the SWDGE optimum: Pool Q7 ucode desc-gen costs ~9-11ns per gathered row
no matter how it's batched (InstDMAGatherAnt = 8.9ns/idx measured, indirect
= 10.9ns/row), so 2176 rows/core ~= 24us, period. The batched-dma_gather
two-stage design (v5) doubled the element count and lost.

What v6 changes vs v4 is the compute, restructured so every candidate-pair
chunk RETIRES COMPLETELY (through its y-contributions) within one gather
cadence, in the transposed position-major domain:
  TT[pos,k]=tanh(B@AM^T) per chunk via matmul(lhsT=BT_chunk, rhs=AMT);
  rows-numerators R^T[k, pair] on PE (lhsT=TT_chunk, rhs=0/1 mask2) instead
  of v4's wide DVE grouped reduces; cols-numerators EC via a 64-col DVE
  reduce; per-chunk exps straight into the LT checkerboard; newB, (AW)^T-
  weighted rows term, and PZ products all per-chunk. After the last gather
  only ONE chunk's short chain + z/y remains (~2.3us tail vs v4's 7.8us).
bf16 pipeline, fp32 string branch with DVE Newton rsqrt (exp/tanh stay the
sole ACT table set), v4's host-side packing.
"""

import numpy as np

V, D, K, C, DS = 500000, 128, 64, 256, 200
NCORES = 8
CC = C // NCORES  # 32 candidates per core
NP = CC // 2      # 16 candidate-pair blocks
NB = NP + 1       # + 1 block for A (t1_ctx rows)
GAMMA = 0.5

_BUILT = None


def _build_nc():
    import concourse.bacc as bacc
    import concourse.mybir as mybir
    from concourse import bass
    from concourse.tile import TileContext

    f32 = mybir.dt.float32
    bf16 = mybir.dt.bfloat16
    i32 = mybir.dt.int32
    AF = mybir.ActivationFunctionType
    ALU = mybir.AluOpType
    AX = mybir.AxisListType

    nc = bacc.Bacc("TRN2", debug=False)

    table_d = nc.dram_tensor("table", (V, D), bf16, kind="ExternalInput")
    idx_d = nc.dram_tensor("idx", (128, NB), i32, kind="ExternalInput")
    # packed bf16 params: att | ident | w  -> [128, 384]
    pk_d = nc.dram_tensor("pk", (128, 3 * 128), bf16, kind="ExternalInput")
    # packed fp32 smalls: str1 | str2 | b -> [CC, 2*DS+1]
    sm_d = nc.dram_tensor("sm", (CC, 2 * DS + 1), f32, kind="ExternalInput")
    y_d = nc.dram_tensor("y", (CC, 1), f32, kind="ExternalOutput")

    GMS = 0.0014  # HW per-gather cadence floor (ms)

    with TileContext(nc) as tc:
        with (
            tc.tile_pool(name="pers", bufs=1) as pp,
            tc.tile_pool(name="btp", bufs=2) as btp,
            tc.tile_pool(name="ps_bt", bufs=2, space="PSUM") as ps_bt,
            tc.tile_pool(name="ps_tt", bufs=2, space="PSUM") as ps_tt,
            tc.tile_pool(name="ps_sm", bufs=2, space="PSUM") as ps_sm,
            tc.tile_pool(name="ps_acc", bufs=1, space="PSUM") as ps_acc,
        ):
            # ---- persistent SBUF tiles ----
            idx_sb = pp.tile([128, NB], i32, tag="idx")
            BG = pp.tile([128, NB * 128], bf16, tag="bg")   # gathered rows
            pk_sb = pp.tile([128, 3 * 128], bf16, tag="pk")
            att_sb = pk_sb[:, 0:128]
            ident = pk_sb[:, 128:256]
            w_sb = pk_sb[:, 256:384]
            sm_sb = pp.tile([CC, 2 * DS + 1], f32, tag="sm")
            str1_sb = sm_sb[:, 0:DS]
            str2_sb = sm_sb[:, DS : 2 * DS]
            b_sb = sm_sb[:, 2 * DS : 2 * DS + 1]

            TT_sb = pp.tile([128, NP * K], bf16, tag="tt")
            ECall = pp.tile([128, NP], f32, tag="ec")
            LT = pp.tile([128, CC], bf16, tag="lt")
            ET2 = pp.tile([K, CC], bf16, tag="et2")
            mask2 = pp.tile([128, 2], bf16, tag="mask2")
            VBT_sb = pp.tile([128, CC], f32, tag="vbt")
            PZ_sb = pp.tile([128, CC], bf16, tag="pz")

            A_T_sb = pp.tile([128, K], bf16, tag="at")
            AMT_sb = pp.tile([128, K], bf16, tag="amt")
            AW_sb = pp.tile([K, 128], bf16, tag="aw")

            ones128b = pp.tile([128, 1], bf16, tag="onesb128")
            ones64b = pp.tile([K, 1], bf16, tag="onesb")
            scr200 = pp.tile([CC, DS], f32, tag="scr200")
            s12_sb = pp.tile([CC, 1], f32, tag="s12")
            s2_sb2 = pp.tile([CC, 1], f32, tag="s2c")
            r12_sb = pp.tile([CC, 1], f32, tag="r12")
            dot_sb = pp.tile([CC, 1], f32, tag="dot")
            ssq2_sb = pp.tile([CC, 1], f32, tag="ssq2")
            ssq1_sb = pp.tile([CC, 1], f32, tag="ssq1")
            den2_sb = pp.tile([CC, 1], f32, tag="den2")
            den_sb = pp.tile([CC, 1], f32, tag="den")
            rden_sb = pp.tile([CC, 1], f32, tag="rden")
            strs_sb = pp.tile([CC, 1], f32, tag="strs")
            sbh_sb = pp.tile([CC, 1], f32, tag="sbh")
            nwt = pp.tile([CC, 1], f32, tag="nwt")
            y_sb = pp.tile([CC, 1], f32, tag="y")

            # ---- input DMAs (idx first: the gather stream waits on it) ----
            nc.sync.dma_start(out=idx_sb[:, :], in_=idx_d[:, :])
            nc.scalar.dma_start(out=pk_sb[:, :], in_=pk_d[:, :])
            nc.sync.dma_start(out=sm_sb[:, :], in_=sm_d[:, :])

            # ---- gathers: A block first (AMT feeds everything), then B.
            # Nothing else runs on Pool, so these stream back-to-back. ----
            def gather(j):
                nc.gpsimd.indirect_dma_start(
                    out=BG[:, 128 * j : 128 * (j + 1)],
                    out_offset=None,
                    in_=table_d[:, :],
                    in_offset=bass.IndirectOffsetOnAxis(
                        ap=idx_sb[:, j : j + 1], axis=0
                    ),
                )

            with tc.tile_wait_until(0.0):
                gather(NP)
            for j in range(NP):
                with tc.tile_wait_until(GMS * (j + 1)):
                    gather(j)

            # ---- constants ----
            nc.vector.memset(ones128b[:, :], 1.0)
            nc.vector.memset(ones64b[:, :], 1.0)
            nc.vector.memset(LT[:, :], 0.0)
            nc.vector.memset(mask2[:, :], 0.0)
            nc.vector.memset(mask2[0:64, 0:1], 1.0)
            nc.vector.memset(mask2[64:128, 1:2], 1.0)

            # ---- string branch on DVE while gathers stream; rsqrt via
            # prescaled Newton (keeps ACT on the exp/tanh table set) ----
            nc.vector.tensor_tensor(out=scr200[:, :], in0=str2_sb[:, :],
                                    in1=str1_sb[:, :], op=ALU.mult)
            nc.vector.reduce_sum(dot_sb[:, :], scr200[:, :], axis=AX.X)
            nc.vector.tensor_tensor(out=scr200[:, :], in0=str2_sb[:, :],
                                    in1=str2_sb[:, :], op=ALU.mult)
            nc.vector.reduce_sum(ssq2_sb[:, :], scr200[:, :], axis=AX.X)
            nc.vector.tensor_tensor(out=scr200[:, :], in0=str1_sb[:, :],
                                    in1=str1_sb[:, :], op=ALU.mult)
            nc.vector.reduce_sum(ssq1_sb[:, :], scr200[:, :], axis=AX.X)
            nc.vector.tensor_tensor(out=den2_sb[:, :], in0=ssq1_sb[:, :],
                                    in1=ssq2_sb[:, :], op=ALU.mult)
            SCL = 1.0 / 40000.0
            nc.vector.tensor_scalar(out=den_sb[:, :], in0=den2_sb[:, :],
                                    scalar1=SCL, scalar2=None, op0=ALU.mult)
            nc.vector.memset(rden_sb[:, :], 1.0)
            for _ in range(5):
                nc.vector.tensor_tensor(out=nwt[:, :], in0=rden_sb[:, :],
                                        in1=rden_sb[:, :], op=ALU.mult)
                nc.vector.tensor_tensor(out=nwt[:, :], in0=nwt[:, :],
                                        in1=den_sb[:, :], op=ALU.mult)
                nc.vector.tensor_scalar(out=nwt[:, :], in0=nwt[:, :],
                                        scalar1=-0.5, scalar2=1.5,
                                        op0=ALU.mult, op1=ALU.add)
                nc.vector.tensor_tensor(out=rden_sb[:, :], in0=rden_sb[:, :],
                                        in1=nwt[:, :], op=ALU.mult)
            nc.vector.tensor_scalar(out=rden_sb[:, :], in0=rden_sb[:, :],
                                    scalar1=1.0 / 200.0, scalar2=None,
                                    op0=ALU.mult)
            nc.vector.tensor_tensor(out=strs_sb[:, :], in0=dot_sb[:, :],
                                    in1=rden_sb[:, :], op=ALU.mult)
            nc.vector.tensor_scalar(out=sbh_sb[:, :], in0=strs_sb[:, :],
                                    scalar1=b_sb[:, 0:1], scalar2=GAMMA,
                                    op0=ALU.add, op1=ALU.mult)

            # ---- A prep: A_T = A^T; AMT = (A@att)^T; AW = A@W ----
            A_sb = BG[0:64, 128 * NP : 128 * NP + 128]  # [K, D] t1 rows
            tc.tile_set_cur_wait(GMS + 0.001)
            A_T_p = ps_sm.tile([128, K], bf16, tag="sm", bufs=2)
            nc.tensor.transpose(A_T_p[:, :], A_sb, ident[0:64, 0:64])
            nc.scalar.copy(A_T_sb[:, :], A_T_p[:, :])
            AMT_p = ps_sm.tile([128, K], f32, tag="sm", bufs=2)
            nc.tensor.matmul(AMT_p[:, :], lhsT=att_sb, rhs=A_T_sb[:, :],
                             start=True, stop=True)
            nc.scalar.copy(AMT_sb[:, :], AMT_p[:, :])
            AW_p = ps_sm.tile([K, 128], f32, tag="sm", bufs=2)
            nc.tensor.matmul(AW_p[:, :], lhsT=A_T_sb[:, :], rhs=w_sb,
                             start=True, stop=True)
            nc.scalar.copy(AW_sb[:, :], AW_p[:, :])

            # ---- persistent PSUM accumulators (one shared bank) ----
            # col layout: RT [0:32) (rows 0:64), VBT [32:64), T1u [64:96),
            # s1/s2/z cols 96/97/98 (rows 0:32)
            acc = ps_acc.tile([128, 128], f32, tag="acc", bufs=1)

            # ---- per-chunk pipeline: each gather block retires fully ----
            for t in range(NP):
                tc.tile_set_cur_wait(GMS * (t + 2) + 0.0015)
                bgc = BG[:, 128 * t : 128 * (t + 1)]
                ttc = TT_sb[:, K * t : K * (t + 1)]
                BT_p = ps_bt.tile([128, 128], bf16, tag="btp", name="bt_p")
                nc.tensor.transpose(BT_p[:, :], bgc, ident)
                btc = btp.tile([128, 128], bf16, tag="btc", name="bt_c")
                if t % 2 == 0:
                    nc.vector.tensor_copy(btc[:, :], BT_p[:, :])
                else:
                    nc.scalar.copy(btc[:, :], BT_p[:, :])
                TT_p = ps_tt.tile([128, K], f32, tag="ttp", name="tt_p")
                nc.tensor.matmul(TT_p[:, :], lhsT=btc[:, :], rhs=AMT_sb[:, :],
                                 start=True, stop=True)
                nc.scalar.activation(ttc, TT_p[:, :], AF.Tanh)
                # cols numerators + weights into the LT checkerboard
                nc.vector.reduce_sum(ECall[:, t : t + 1], ttc, axis=AX.X)
                nc.scalar.activation(LT[0:64, 2 * t : 2 * t + 1],
                                     ECall[0:64, t : t + 1],
                                     AF.Exp, scale=1.0 / K)
                nc.scalar.activation(LT[64:128, 2 * t + 1 : 2 * t + 2],
                                     ECall[64:128, t : t + 1],
                                     AF.Exp, scale=1.0 / K)
                # rows numerators on PE
                nc.tensor.matmul(acc[0:K, 2 * t : 2 * t + 2], lhsT=ttc,
                                 rhs=mask2[:, :], start=True, stop=True)
                # newB pair (unnormalized)
                nc.tensor.matmul(acc[:, 32 + 2 * t : 34 + 2 * t],
                                 lhsT=bgc, rhs=LT[:, 2 * t : 2 * t + 2],
                                 start=True, stop=True)
                nc.vector.tensor_copy(VBT_sb[:, 2 * t : 2 * t + 2],
                                      acc[:, 32 + 2 * t : 34 + 2 * t])
                # rows weights + (A@W)^T-weighted term for this pair
                nc.scalar.activation(ET2[:, 2 * t : 2 * t + 2],
                                     acc[0:K, 2 * t : 2 * t + 2],
                                     AF.Exp, scale=1.0 / K)
                nc.tensor.matmul(acc[:, 64 + 2 * t : 66 + 2 * t],
                                 lhsT=AW_sb[:, :],
                                 rhs=ET2[:, 2 * t : 2 * t + 2],
                                 start=True, stop=True)
                nc.vector.tensor_tensor(out=PZ_sb[:, 2 * t : 2 * t + 2],
                                        in0=acc[:, 64 + 2 * t : 66 + 2 * t],
                                        in1=VBT_sb[:, 2 * t : 2 * t + 2],
                                        op=ALU.mult)

            # ---- softmax denominators (overlap the last chunks) ----
            tc.tile_set_cur_wait(GMS * 16 + 0.003)
            nc.tensor.matmul(acc[0:CC, 97:98], lhsT=LT[:, :],
                             rhs=ones128b[:, :], start=True, stop=True)
            nc.tensor.matmul(acc[0:CC, 96:97], lhsT=ET2[:, :],
                             rhs=ones64b[:, :], start=True, stop=True)
            nc.vector.tensor_scalar(out=s2_sb2[:, :], in0=acc[0:CC, 97:98],
                                    scalar1=1.0 / GAMMA, scalar2=None,
                                    op0=ALU.mult)
            nc.vector.tensor_tensor(out=s12_sb[:, :], in0=acc[0:CC, 96:97],
                                    in1=s2_sb2[:, :], op=ALU.mult)
            nc.vector.reciprocal(r12_sb[:, :], s12_sb[:, :])

            # ---- bilinear reduce + y = z*r12 + 0.5*(str + b) ----
            tc.tile_set_cur_wait(GMS * 17 + 0.0035)
            nc.tensor.matmul(acc[0:CC, 98:99], lhsT=PZ_sb[:, :],
                             rhs=ones128b[:, :], start=True, stop=True)
            nc.vector.tensor_scalar(out=y_sb[:, :], in0=acc[0:CC, 98:99],
                                    scalar1=r12_sb[:, 0:1],
                                    scalar2=sbh_sb[:, 0:1],
                                    op0=ALU.mult, op1=ALU.add)

            nc.sync.dma_start(out=y_d[:, :], in_=y_sb[:, :])

    nc.compile()
    return nc


def get_nc():
    global _BUILT
    if _BUILT is None:
        _BUILT = _build_nc()
    return _BUILT


def make_in_maps(table, str_t1, str_t2s, att_mat, W_bi, b_bi, t1_ctx, t2_ctx):
    import ml_dtypes

    table = np.asarray(table, dtype=np.float32)
    str_t1 = np.asarray(str_t1, dtype=np.float32).reshape(DS)
    str_t2s = np.asarray(str_t2s, dtype=np.float32)
    att_mat = np.asarray(att_mat, dtype=np.float32)
    w2d = np.asarray(W_bi, dtype=np.float32).reshape(D, D)
    bval = float(np.asarray(b_bi).reshape(-1)[0])
    t1 = np.asarray(t1_ctx).astype(np.int32)
    t2 = np.asarray(t2_ctx).astype(np.int32)

    table_bf = table.astype(ml_dtypes.bfloat16)
    pk = np.concatenate(
        [att_mat, np.eye(D, dtype=np.float32), w2d], axis=1
    ).astype(ml_dtypes.bfloat16)  # [128, 384]

    sm = np.empty((CC, 2 * DS + 1), np.float32)
    sm[:, 0:DS] = str_t1[None, :]
    sm[:, 2 * DS] = bval

    in_maps = []
    for i in range(NCORES):
        c0 = i * CC
        t2s = t2[c0 : c0 + CC]  # [CC, K]
        idx = np.empty((128, NB), np.int32)
        idx[0:64, 0:NP] = t2s[0::2, :].T   # even candidates on partitions 0-63
        idx[64:128, 0:NP] = t2s[1::2, :].T  # odd candidates on partitions 64-127
        idx[0:64, NP] = t1
        idx[64:128, NP] = t1
        smc = sm.copy()
        smc[:, DS : 2 * DS] = str_t2s[c0 : c0 + CC]
        in_maps.append({
            "table": table_bf,
            "idx": idx,
            "pk": pk,
            "sm": smc,
        })
    return in_maps


def run(inputs: dict, trace: bool = False):
    from concourse.bass_utils import run_bass_kernel_spmd

    nc = get_nc()
    in_maps = make_in_maps(**inputs)
    res = run_bass_kernel_spmd(
        nc, in_maps, core_ids=list(range(NCORES)), trace=trace
    )
    y = np.concatenate([r["y"].reshape(-1) for r in res.results])
    return y.reshape(1, C).astype(np.float32), res


def kernel(**inputs) -> np.ndarray:
    y, _ = run(inputs, trace=False)
    return y


# revision 26
# speedup vs baseline: 1.5607x; 1.0898x over previous
"""DeepTermRankingListNet Trainium2 kernel.

Full-input contract: kernel(**inputs) takes the unsharded numpy inputs and
returns the full [1, 256] output. Internally shards candidates C=256 across
8 NeuronCores (32 each), replicates the embedding table + small params,
runs one SPMD Bass/Tile kernel via run_bass_kernel_spmd, and concatenates
the per-core [32] outputs.

v6. The gather stream is v4's: 17 indirect_dma_start calls (A block first,
then 16 candidate-pair blocks, one row per partition), which probing showed
is the SWDGE optimum: Pool Q7 ucode desc-gen costs ~9-11ns per gathered row
no matter how it's batched (InstDMAGatherAnt = 8.9ns/idx measured, indirect
= 10.9ns/row), so 2176 rows/core ~= 24us, period. The batched-dma_gather
two-stage design (v5) doubled the element count and lost.

What v6 changes vs v4 is the compute, restructured so every candidate-pair
chunk RETIRES COMPLETELY (through its y-contributions) within one gather
cadence, in the transposed position-major domain:
  TT[pos,k]=tanh(B@AM^T) per chunk via matmul(lhsT=BT_chunk, rhs=AMT);
  rows-numerators R^T[k, pair] on PE (lhsT=TT_chunk, rhs=0/1 mask2) instead
  of v4's wide DVE grouped reduces; cols-numerators EC via a 64-col DVE
  reduce; per-chunk exps straight into the LT checkerboard; newB, (AW)^T-
  weighted rows term, and PZ products all per-chunk. After the last gather
  only ONE chunk's short chain + z/y remains (~2.3us tail vs v4's 7.8us).
bf16 pipeline, fp32 string branch with DVE Newton rsqrt (exp/tanh stay the
sole ACT table set), v4's host-side packing.
"""

import numpy as np

V, D, K, C, DS = 500000, 128, 64, 256, 200
NCORES = 8
CC = C // NCORES  # 32 candidates per core
NP = CC // 2      # 16 candidate-pair blocks
NB = NP + 1       # + 1 block for A (t1_ctx rows)
GAMMA = 0.5

_BUILT = None


def _build_nc():
    import concourse.bacc as bacc
    import concourse.mybir as mybir
    from concourse import bass
    from concourse.tile import TileContext

    f32 = mybir.dt.float32
    bf16 = mybir.dt.bfloat16
    i32 = mybir.dt.int32
    AF = mybir.ActivationFunctionType
    ALU = mybir.AluOpType
    AX = mybir.AxisListType

    nc = bacc.Bacc("TRN2", debug=False)

    table_d = nc.dram_tensor("table", (V, D), bf16, kind="ExternalInput")
    idx_d = nc.dram_tensor("idx", (128, NB), i32, kind="ExternalInput")
    # packed bf16 params: att | ident | w  -> [128, 384]
    pk_d = nc.dram_tensor("pk", (128, 3 * 128), bf16, kind="ExternalInput")
    # packed fp32 smalls: str1 | str2 | b -> [CC, 2*DS+1]
    sm_d = nc.dram_tensor("sm", (CC, 2 * DS + 1), f32, kind="ExternalInput")
    y_d = nc.dram_tensor("y", (CC, 1), f32, kind="ExternalOutput")

    GMS = 0.0014  # HW per-gather cadence floor (ms)

    with TileContext(nc) as tc:
        with (
            tc.tile_pool(name="pers", bufs=1) as pp,
            tc.tile_pool(name="btp", bufs=2) as btp,
            tc.tile_pool(name="ps_bt", bufs=2, space="PSUM") as ps_bt,
            tc.tile_pool(name="ps_tt", bufs=2, space="PSUM") as ps_tt,
            tc.tile_pool(name="ps_sm", bufs=2, space="PSUM") as ps_sm,
            tc.tile_pool(name="ps_acc", bufs=1, space="PSUM") as ps_acc,
        ):
            # ---- persistent SBUF tiles ----
            idx_sb = pp.tile([128, NB], i32, tag="idx")
            BG = pp.tile([128, NB * 128], bf16, tag="bg")   # gathered rows
            pk_sb = pp.tile([128, 3 * 128], bf16, tag="pk")
            att_sb = pk_sb[:, 0:128]
            ident = pk_sb[:, 128:256]
            w_sb = pk_sb[:, 256:384]
            sm_sb = pp.tile([CC, 2 * DS + 1], f32, tag="sm")
            str1_sb = sm_sb[:, 0:DS]
            str2_sb = sm_sb[:, DS : 2 * DS]
            b_sb = sm_sb[:, 2 * DS : 2 * DS + 1]

            TT_sb = pp.tile([128, NP * K], bf16, tag="tt")
            ECall = pp.tile([128, NP], f32, tag="ec")
            LT = pp.tile([128, CC], bf16, tag="lt")
            ET2 = pp.tile([K, CC], bf16, tag="et2")
            mask2 = pp.tile([128, 2], bf16, tag="mask2")
            VBT_sb = pp.tile([128, CC], f32, tag="vbt")
            PZ_sb = pp.tile([128, CC], bf16, tag="pz")

            A_T_sb = pp.tile([128, K], bf16, tag="at")
            AMT_sb = pp.tile([128, K], bf16, tag="amt")
            AW_sb = pp.tile([K, 128], bf16, tag="aw")

            ones128b = pp.tile([128, 1], bf16, tag="onesb128")
            ones64b = pp.tile([K, 1], bf16, tag="onesb")
            scr200 = pp.tile([CC, DS], f32, tag="scr200")
            s12_sb = pp.tile([CC, 1], f32, tag="s12")
            s2_sb2 = pp.tile([CC, 1], f32, tag="s2c")
            r12_sb = pp.tile([CC, 1], f32, tag="r12")
            dot_sb = pp.tile([CC, 1], f32, tag="dot")
            ssq2_sb = pp.tile([CC, 1], f32, tag="ssq2")
            ssq1_sb = pp.tile([CC, 1], f32, tag="ssq1")
            den2_sb = pp.tile([CC, 1], f32, tag="den2")
            den_sb = pp.tile([CC, 1], f32, tag="den")
            rden_sb = pp.tile([CC, 1], f32, tag="rden")
            strs_sb = pp.tile([CC, 1], f32, tag="strs")
            sbh_sb = pp.tile([CC, 1], f32, tag="sbh")
            nwt = pp.tile([CC, 1], f32, tag="nwt")
            y_sb = pp.tile([CC, 1], f32, tag="y")

            # ---- input DMAs (idx first: the gather stream waits on it) ----
            nc.sync.dma_start(out=idx_sb[:, :], in_=idx_d[:, :])
            nc.scalar.dma_start(out=pk_sb[:, :], in_=pk_d[:, :])
            nc.sync.dma_start(out=sm_sb[:, :], in_=sm_d[:, :])

            # ---- gathers: A block first (AMT feeds everything), then B.
            # Nothing else runs on Pool, so these stream back-to-back. ----
            def gather(j):
                nc.gpsimd.indirect_dma_start(
                    out=BG[:, 128 * j : 128 * (j + 1)],
                    out_offset=None,
                    in_=table_d[:, :],
                    in_offset=bass.IndirectOffsetOnAxis(
                        ap=idx_sb[:, j : j + 1], axis=0
                    ),
                )

            with tc.tile_wait_until(0.0):
                gather(NP)
            for j in range(NP):
                with tc.tile_wait_until(GMS * (j + 1)):
                    gather(j)

            # ---- constants ----
            nc.vector.memset(ones128b[:, :], 1.0)
            nc.vector.memset(ones64b[:, :], 1.0)
            nc.vector.memset(LT[:, :], 0.0)
            nc.vector.memset(mask2[:, :], 0.0)
            nc.vector.memset(mask2[0:64, 0:1], 1.0)
            nc.vector.memset(mask2[64:128, 1:2], 1.0)

            # ---- string branch on DVE while gathers stream; rsqrt via
            # prescaled Newton (keeps ACT on the exp/tanh table set) ----
            nc.vector.tensor_tensor(out=scr200[:, :], in0=str2_sb[:, :],
                                    in1=str1_sb[:, :], op=ALU.mult)
            nc.vector.reduce_sum(dot_sb[:, :], scr200[:, :], axis=AX.X)
            nc.vector.tensor_tensor(out=scr200[:, :], in0=str2_sb[:, :],
                                    in1=str2_sb[:, :], op=ALU.mult)
            nc.vector.reduce_sum(ssq2_sb[:, :], scr200[:, :], axis=AX.X)
            nc.vector.tensor_tensor(out=scr200[:, :], in0=str1_sb[:, :],
                                    in1=str1_sb[:, :], op=ALU.mult)
            nc.vector.reduce_sum(ssq1_sb[:, :], scr200[:, :], axis=AX.X)
            nc.vector.tensor_tensor(out=den2_sb[:, :], in0=ssq1_sb[:, :],
                                    in1=ssq2_sb[:, :], op=ALU.mult)
            SCL = 1.0 / 40000.0
            nc.vector.tensor_scalar(out=den_sb[:, :], in0=den2_sb[:, :],
                                    scalar1=SCL, scalar2=None, op0=ALU.mult)
            nc.vector.memset(rden_sb[:, :], 1.0)
            for _ in range(5):
                nc.vector.tensor_tensor(out=nwt[:, :], in0=rden_sb[:, :],
                                        in1=rden_sb[:, :], op=ALU.mult)
                nc.vector.tensor_tensor(out=nwt[:, :], in0=nwt[:, :],
                                        in1=den_sb[:, :], op=ALU.mult)
                nc.vector.tensor_scalar(out=nwt[:, :], in0=nwt[:, :],
                                        scalar1=-0.5, scalar2=1.5,
                                        op0=ALU.mult, op1=ALU.add)
                nc.vector.tensor_tensor(out=rden_sb[:, :], in0=rden_sb[:, :],
                                        in1=nwt[:, :], op=ALU.mult)
            nc.vector.tensor_scalar(out=rden_sb[:, :], in0=rden_sb[:, :],
                                    scalar1=1.0 / 200.0, scalar2=None,
                                    op0=ALU.mult)
            nc.vector.tensor_tensor(out=strs_sb[:, :], in0=dot_sb[:, :],
                                    in1=rden_sb[:, :], op=ALU.mult)
            nc.vector.tensor_scalar(out=sbh_sb[:, :], in0=strs_sb[:, :],
                                    scalar1=b_sb[:, 0:1], scalar2=GAMMA,
                                    op0=ALU.add, op1=ALU.mult)

            # ---- A prep: A_T = A^T; AMT = (A@att)^T; AW = A@W ----
            A_sb = BG[0:64, 128 * NP : 128 * NP + 128]  # [K, D] t1 rows
            tc.tile_set_cur_wait(GMS + 0.001)
            A_T_p = ps_sm.tile([128, K], bf16, tag="sm", bufs=2)
            nc.tensor.transpose(A_T_p[:, :], A_sb, ident[0:64, 0:64])
            nc.scalar.copy(A_T_sb[:, :], A_T_p[:, :])
            AMT_p = ps_sm.tile([128, K], f32, tag="sm", bufs=2)
            nc.tensor.matmul(AMT_p[:, :], lhsT=att_sb, rhs=A_T_sb[:, :],
                             start=True, stop=True)
            nc.scalar.copy(AMT_sb[:, :], AMT_p[:, :])
            AW_p = ps_sm.tile([K, 128], f32, tag="sm", bufs=2)
            nc.tensor.matmul(AW_p[:, :], lhsT=A_T_sb[:, :], rhs=w_sb,
                             start=True, stop=True)
            nc.scalar.copy(AW_sb[:, :], AW_p[:, :])

            # ---- persistent PSUM accumulators (one shared bank) ----
            # col layout: RT [0:32) (rows 0:64), VBT [32:64), T1u [64:96),
            # s1/s2/z cols 96/97/98 (rows 0:32)
            acc = ps_acc.tile([128, 128], f32, tag="acc", bufs=1)

            # ---- per-chunk pipeline, software-pipelined in 2 phases so the
            # ~2.3us cross-engine dependency chain never throttles the PE
            # queue below the 1.4us gather cadence ----
            def phase_a(t):
                bgc = BG[:, 128 * t : 128 * (t + 1)]
                ttc = TT_sb[:, K * t : K * (t + 1)]
                BT_p = ps_bt.tile([128, 128], bf16, tag="btp", name="bt_p")
                nc.tensor.transpose(BT_p[:, :], bgc, ident)
                btc = btp.tile([128, 128], bf16, tag="btc", name="bt_c")
                if t % 2 == 0:
                    nc.vector.tensor_copy(btc[:, :], BT_p[:, :])
                else:
                    nc.scalar.copy(btc[:, :], BT_p[:, :])
                TT_p = ps_tt.tile([128, K], f32, tag="ttp", name="tt_p")
                nc.tensor.matmul(TT_p[:, :], lhsT=btc[:, :], rhs=AMT_sb[:, :],
                                 start=True, stop=True)
                # tanh + fused cols-numerator row-sum (EC)
                nc.scalar.activation(ttc, TT_p[:, :], AF.Tanh,
                                     accum_out=ECall[:, t : t + 1])
                # cols weights into the LT checkerboard
                nc.scalar.activation(LT[0:64, 2 * t : 2 * t + 1],
                                     ECall[0:64, t : t + 1],
                                     AF.Exp, scale=1.0 / K)
                nc.scalar.activation(LT[64:128, 2 * t + 1 : 2 * t + 2],
                                     ECall[64:128, t : t + 1],
                                     AF.Exp, scale=1.0 / K)
                # rows numerators on PE
                nc.tensor.matmul(acc[0:K, 2 * t : 2 * t + 2], lhsT=ttc,
                                 rhs=mask2[:, :], start=True, stop=True)

            def phase_b(t):
                bgc = BG[:, 128 * t : 128 * (t + 1)]
                # newB pair (unnormalized)
                nc.tensor.matmul(acc[:, 32 + 2 * t : 34 + 2 * t],
                                 lhsT=bgc, rhs=LT[:, 2 * t : 2 * t + 2],
                                 start=True, stop=True)
                nc.vector.tensor_copy(VBT_sb[:, 2 * t : 2 * t + 2],
                                      acc[:, 32 + 2 * t : 34 + 2 * t])
                # rows weights + (A@W)^T-weighted term for this pair
                nc.scalar.activation(ET2[:, 2 * t : 2 * t + 2],
                                     acc[0:K, 2 * t : 2 * t + 2],
                                     AF.Exp, scale=1.0 / K)
                nc.tensor.matmul(acc[:, 64 + 2 * t : 66 + 2 * t],
                                 lhsT=AW_sb[:, :],
                                 rhs=ET2[:, 2 * t : 2 * t + 2],
                                 start=True, stop=True)
                nc.vector.tensor_tensor(out=PZ_sb[:, 2 * t : 2 * t + 2],
                                        in0=acc[:, 64 + 2 * t : 66 + 2 * t],
                                        in1=VBT_sb[:, 2 * t : 2 * t + 2],
                                        op=ALU.mult)

            for t in range(NP):
                tc.tile_set_cur_wait(GMS * (t + 2) + 0.0015)
                phase_a(t)
                if t >= 1:
                    phase_b(t - 1)
            tc.tile_set_cur_wait(GMS * 17 + 0.002)
            phase_b(NP - 1)

            # ---- softmax denominators (overlap the last chunks) ----
            tc.tile_set_cur_wait(GMS * 16 + 0.003)
            nc.tensor.matmul(acc[0:CC, 97:98], lhsT=LT[:, :],
                             rhs=ones128b[:, :], start=True, stop=True)
            nc.tensor.matmul(acc[0:CC, 96:97], lhsT=ET2[:, :],
                             rhs=ones64b[:, :], start=True, stop=True)
            nc.vector.tensor_scalar(out=s2_sb2[:, :], in0=acc[0:CC, 97:98],
                                    scalar1=1.0 / GAMMA, scalar2=None,
                                    op0=ALU.mult)
            nc.vector.tensor_tensor(out=s12_sb[:, :], in0=acc[0:CC, 96:97],
                                    in1=s2_sb2[:, :], op=ALU.mult)
            nc.vector.reciprocal(r12_sb[:, :], s12_sb[:, :])

            # ---- bilinear reduce + y = z*r12 + 0.5*(str + b) ----
            tc.tile_set_cur_wait(GMS * 17 + 0.0035)
            nc.tensor.matmul(acc[0:CC, 98:99], lhsT=PZ_sb[:, :],
                             rhs=ones128b[:, :], start=True, stop=True)
            nc.vector.tensor_scalar(out=y_sb[:, :], in0=acc[0:CC, 98:99],
                                    scalar1=r12_sb[:, 0:1],
                                    scalar2=sbh_sb[:, 0:1],
                                    op0=ALU.mult, op1=ALU.add)

            nc.sync.dma_start(out=y_d[:, :], in_=y_sb[:, :])

    nc.compile()
    return nc


def get_nc():
    global _BUILT
    if _BUILT is None:
        _BUILT = _build_nc()
    return _BUILT


def make_in_maps(table, str_t1, str_t2s, att_mat, W_bi, b_bi, t1_ctx, t2_ctx):
    import ml_dtypes

    table = np.asarray(table, dtype=np.float32)
    str_t1 = np.asarray(str_t1, dtype=np.float32).reshape(DS)
    str_t2s = np.asarray(str_t2s, dtype=np.float32)
    att_mat = np.asarray(att_mat, dtype=np.float32)
    w2d = np.asarray(W_bi, dtype=np.float32).reshape(D, D)
    bval = float(np.asarray(b_bi).reshape(-1)[0])
    t1 = np.asarray(t1_ctx).astype(np.int32)
    t2 = np.asarray(t2_ctx).astype(np.int32)

    table_bf = table.astype(ml_dtypes.bfloat16)
    pk = np.concatenate(
        [att_mat, np.eye(D, dtype=np.float32), w2d], axis=1
    ).astype(ml_dtypes.bfloat16)  # [128, 384]

    sm = np.empty((CC, 2 * DS + 1), np.float32)
    sm[:, 0:DS] = str_t1[None, :]
    sm[:, 2 * DS] = bval

    in_maps = []
    for i in range(NCORES):
        c0 = i * CC
        t2s = t2[c0 : c0 + CC]  # [CC, K]
        idx = np.empty((128, NB), np.int32)
        idx[0:64, 0:NP] = t2s[0::2, :].T   # even candidates on partitions 0-63
        idx[64:128, 0:NP] = t2s[1::2, :].T  # odd candidates on partitions 64-127
        idx[0:64, NP] = t1
        idx[64:128, NP] = t1
        smc = sm.copy()
        smc[:, DS : 2 * DS] = str_t2s[c0 : c0 + CC]
        in_maps.append({
            "table": table_bf,
            "idx": idx,
            "pk": pk,
            "sm": smc,
        })
    return in_maps


def run(inputs: dict, trace: bool = False):
    from concourse.bass_utils import run_bass_kernel_spmd

    nc = get_nc()
    in_maps = make_in_maps(**inputs)
    res = run_bass_kernel_spmd(
        nc, in_maps, core_ids=list(range(NCORES)), trace=trace
    )
    y = np.concatenate([r["y"].reshape(-1) for r in res.results])
    return y.reshape(1, C).astype(np.float32), res


def kernel(**inputs) -> np.ndarray:
    y, _ = run(inputs, trace=False)
    return y
